# revision 41
# baseline (speedup 1.0000x reference)
"""Expert-choice MoE FFN on 8 trn2 cores — host routing + int8-wire device GEMM.

The torch module reuses ONE shared expert Linear for all 16 experts, so
the grouped expert GEMM collapses to a per-token scale of a single GEMM:
    y[t] = coeff[t] * (x[t] @ W + b),
    coeff[t] = sum_e S[t,e] * [S[t,e] >= theta_e]
where S = softmax(x @ Wr + br) and theta_e is the 512th-largest value of
softmax column e (expert-choice top-k).

The axon tunnel to the devices moves ~40-50 MB/s half-duplex, so wire
bytes dominate wall time; device compute (68.7 GFLOP GEMM) is
sub-millisecond.  Split:
  - routing (0.5 GFLOP) runs on host in fp32 with fp64 refinement of
    near-threshold rows, which reproduces the reference's fp32 top-k
    selection exactly;
  - x ships as int8 with a per-token scale s_t (host round-half-up;
    the row scale commutes through the GEMM).  The bias rides the GEMM
    as an extra rank-1 matmul with lhsT = 1/s_t so it lands in psum
    before the final scale;
  - each core dequantizes to f16, PE-transposes its tiles, runs the
    f16 GEMM against resident f16 weights, then quantizes y back to
    int8 with a per-token scale computed on-device, packed into a
    trailing row of the y8 tensor (single fetch);
  - expert weights upload once (1/8th per core + on-device all_gather)
    and stay device-resident across calls, keyed by content;
  - the call is split into two half-token pipelined executions so the
    device exec and host quant/dequant hide under the half-duplex wire
    transfers.  Warm calls pay x8-in (16.8 MB) + y8-out (16.9 MB).
"""

import numpy as np

NCORES = 8
BS, H, E, KSEL = 8192, 2048, 16, 512
TPC = BS // NCORES          # 1024 tokens per core
NCHUNK = 2
CTPC = TPC // NCHUNK        # 512 tokens per core per chunk
CMT = CTPC // 128           # 4 m-tiles per chunk
KS = H // 128               # 16 k-slabs
QCAP = 126.5                # int8 target magnitude; <127 guards saturation

_G = {}


# ---------------------------------------------------------------- routing
def _routing_coeff(xf, router_w, router_b):
    """coeff[t] = sum of gates of experts that picked token t.

    fp32 BLAS for the bulk, then fp64 refinement of the few rows whose
    gate sits within 2e-4 of an expert threshold, so the top-k selection
    matches exact arithmetic (and thus the fp32 reference) while costing
    ~1/6 of a full fp64 pass on the single host CPU."""
    lg = xf @ router_w
    lg += router_b[None, :]
    lg -= lg.max(axis=1, keepdims=True)
    ex = np.exp(lg)
    S = ex / ex.sum(axis=1, keepdims=True)            # (BS, E)
    theta = np.partition(S, BS - KSEL, axis=0)[BS - KSEL, :]
    dmin = np.abs(S - theta[None, :]).min(axis=1)
    idx = np.nonzero(dmin < 2e-4)[0]
    if idx.size:
        lg64 = xf[idx].astype(np.float64) @ router_w.astype(np.float64)
        lg64 += router_b.astype(np.float64)[None, :]
        lg64 -= lg64.max(axis=1, keepdims=True)
        e64 = np.exp(lg64)
        S[idx] = (e64 / e64.sum(axis=1, keepdims=True)).astype(np.float32)
        theta = np.partition(S, BS - KSEL, axis=0)[BS - KSEL, :]
    return ((S * (S >= theta[None, :])).sum(axis=1)).astype(np.float32)


# ---------------------------------------------------------------- device body
def _body(tc, x8, cs, invs, w, bvec, ident, y8, mt):
    import concourse.mybir as mybir
    from concourse.bass import ts

    f16, f32 = mybir.dt.float16, mybir.dt.float32
    i8 = mybir.dt.int8
    ACT = mybir.ActivationFunctionType
    ALU = mybir.AluOpType
    X = mybir.AxisListType.X
    nc = tc.nc
    tpc = mt * 128
    with (
        tc.tile_pool(name="wtp", bufs=KS) as wtp,
        tc.tile_pool(name="x8p", bufs=2) as x8p,
        tc.tile_pool(name="xsp", bufs=2) as xsp,
        tc.tile_pool(name="xtp", bufs=2) as xtp,
        tc.tile_pool(name="sbp", bufs=1) as sbp,
        tc.tile_pool(name="rdp", bufs=2) as rdp,
        tc.tile_pool(name="outp", bufs=8) as outp,
        tc.tile_pool(name="pp", bufs=5, space="PSUM") as pp,
        tc.tile_pool(name="ptp", bufs=2, space="PSUM") as ptp,
    ):
        # resident weights: 16 slabs [128k, 2048n] f16 (8.4 MB)
        wts = []
        for k in range(KS):
            wt = wtp.tile([128, H], f16, name=f"w{k}", tag="wt")
            nc.sync.dma_start(wt, w[ts(k, 128), :])
            wts.append(wt)
        bvec_sb = sbp.tile([1, H], f16)
        nc.sync.dma_start(bvec_sb, bvec)
        ident_sb = sbp.tile([128, 128], f16)
        nc.sync.dma_start(ident_sb, ident)
        cs_sb = sbp.tile([128, mt], f32)
        nc.sync.dma_start(cs_sb, cs)
        invs_sb = sbp.tile([1, tpc], f16)
        nc.sync.dma_start(invs_sb, invs)
        h_sb = sbp.tile([128, mt], f32)

        for m in range(mt):
            # int8 x tile -> f16, then PE-transpose per k-block
            xs8 = x8p.tile([128, H], i8, name=f"x8_{m}", tag="x8")
            nc.sync.dma_start(xs8, x8[ts(m, 128), :])
            xs = xsp.tile([128, H], f16, name=f"xs{m}", tag="xs")
            nc.vector.tensor_copy(xs, xs8)
            xT = xtp.tile([128, KS * 128], f16, name=f"xT{m}", tag="xT")
            for k in range(KS):
                tp = ptp.tile([128, 128], f16, tag="tp")
                nc.tensor.transpose(tp, xs[:, ts(k, 128)], ident_sb)
                nc.vector.tensor_copy(xT[:, ts(k, 128)], tp)

            pmms = []
            rmax = rdp.tile([128, 4], f32, name=f"rm{m}", tag="rm")
            for nj in range(4):
                pmm = pp.tile([128, 512], f32, name=f"mm{m}_{nj}", tag="mm")
                for k in range(KS):
                    nc.tensor.matmul(pmm, xT[:, ts(k, 128)],
                                     wts[k][:, ts(nj, 512)],
                                     start=(k == 0), stop=False)
                nc.tensor.matmul(pmm, invs_sb[0:1, ts(m, 128)],
                                 bvec_sb[0:1, ts(nj, 512)],
                                 start=False, stop=True)
                mx = rdp.tile([128, 1], f32, tag="mx")
                nc.vector.tensor_reduce(mx, pmm, axis=X, op=ALU.max)
                nmn = rdp.tile([128, 1], f32, tag="nmn")
                nc.vector.tensor_reduce(nmn, pmm, axis=X, op=ALU.min,
                                        negate=True)
                nc.vector.tensor_tensor(rmax[:, nj:nj + 1], mx, nmn,
                                        op=ALU.max)
                pmms.append(pmm)

            # per-token scales: rmz = absmax(psum row) * cs
            rm = rdp.tile([128, 1], f32, tag="rm1")
            nc.vector.tensor_reduce(rm, rmax, axis=X, op=ALU.max)
            rmz = rdp.tile([128, 1], f32, tag="rmz")
            nc.vector.tensor_tensor(rmz, rm, cs_sb[:, m:m + 1], op=ALU.mult)
            cl = rdp.tile([128, 1], f32, tag="cl")
            nc.vector.tensor_scalar(cl, rmz, 1e-20, None, op0=ALU.max)
            nc.vector.tensor_scalar_mul(h_sb[:, m:m + 1], cl, 1.0 / QCAP)
            rec = rdp.tile([128, 1], f32, tag="rec")
            nc.vector.reciprocal(rec, cl)
            g = rdp.tile([128, 1], f32, tag="g")
            nc.vector.tensor_tensor(g, rec, cs_sb[:, m:m + 1], op=ALU.mult)
            gq = rdp.tile([128, 1], f32, tag="gq")
            nc.vector.tensor_scalar_mul(gq, g, QCAP)

            for nj in range(4):
                yo = outp.tile([128, 512], i8, tag="yo")
                nc.scalar.activation(yo, pmms[nj], ACT.Copy, scale=gq)
                nc.sync.dma_start(y8[ts(m, 128), ts(nj, 512)], yo)

        # pack per-token dequant scales h (f32 [128, mt] = mt*512 B) into
        # trailing int8 rows of y8 so the host needs a single fetch
        nrow = (mt * 512 + H - 1) // H
        hdst = y8[tpc:tpc + nrow, :].bitcast(f32).rearrange(
            "a (b m) -> (a b) m", m=mt)
        nc.sync.dma_start(hdst, h_sb)


def _build():
    if "nc" in _G:
        return _G["nc"]
    import concourse.mybir as mybir
    import concourse.bacc as bacc
    import concourse.tile as tile

    f16, f32 = mybir.dt.float16, mybir.dt.float32
    i8 = mybir.dt.int8
    nrow = (CMT * 512 + H - 1) // H
    nc = bacc.Bacc("TRN2", target_bir_lowering=False, debug=False,
                   num_devices=1)
    x8 = nc.dram_tensor("x8", [CTPC, H], i8, kind="ExternalInput").ap()
    cs = nc.dram_tensor("cs", [128, CMT], f32, kind="ExternalInput").ap()
    invs = nc.dram_tensor("invs", [1, CTPC], f16, kind="ExternalInput").ap()
    w = nc.dram_tensor("w", [H, H], f16, kind="ExternalInput").ap()
    bvec = nc.dram_tensor("bvec", [1, H], f16, kind="ExternalInput").ap()
    ident = nc.dram_tensor("ident", [128, 128], f16,
                           kind="ExternalInput").ap()
    y8 = nc.dram_tensor("y8", [CTPC + nrow, H], i8,
                        kind="ExternalOutput").ap()
    with tile.TileContext(nc) as tc:
        _body(tc, x8, cs, invs, w, bvec, ident, y8, CMT)
    nc.compile()
    _G["nc"] = nc
    return nc


# ---------------------------------------------------------------- runtime
def _runtime():
    if "rt" in _G:
        return _G["rt"]
    from concurrent.futures import ThreadPoolExecutor

    import jax
    import jax.numpy as jnp
    from jax.experimental.shard_map import shard_map
    from jax.sharding import Mesh, NamedSharding, PartitionSpec

    import concourse.mybir as mybir
    from concourse import bass2jax

    nc = _build()
    bass2jax.install_neuronx_cc_hook()

    partition_name = (nc.partition_id_tensor.name
                      if nc.partition_id_tensor else None)
    in_names, out_names, out_avals, zero_shapes = [], [], [], []
    for alloc in nc.m.functions[0].allocations:
        if not isinstance(alloc, mybir.MemoryLocationSet):
            continue
        name = alloc.memorylocations[0].name
        if alloc.kind == "ExternalInput":
            if name != partition_name:
                in_names.append(name)
        elif alloc.kind == "ExternalOutput":
            out_names.append(name)
            shape = tuple(alloc.tensor_shape)
            dtype = mybir.dt.np(alloc.dtype)
            out_avals.append(jax.core.ShapedArray(shape, dtype))
            zero_shapes.append((shape, dtype))
    n_in, n_out = len(in_names), len(out_names)
    # the kernel writes every output byte, so outputs need not be bound
    # as (donated, pre-zeroed) inputs — PJRT allocates them as results
    all_in_names = list(in_names)
    if partition_name is not None:
        all_in_names.append(partition_name)

    devices = jax.devices()[:NCORES]
    mesh = Mesh(np.asarray(devices), ("core",))
    shard = NamedSharding(mesh, PartitionSpec("core"))

    def _exec_body(*args):
        operands = list(args)
        if partition_name is not None:
            operands.append(bass2jax.partition_id_tensor())
        return tuple(bass2jax._bass_exec_p.bind(
            *operands,
            out_avals=tuple(out_avals),
            in_names=tuple(all_in_names),
            out_names=tuple(out_names),
            lowering_input_output_aliases=(),
            sim_require_finite=True,
            sim_require_nnan=True,
            nc=nc,
        ))

    sharded = jax.jit(
        shard_map(_exec_body, mesh=mesh,
                  in_specs=(PartitionSpec("core"),) * n_in,
                  out_specs=(PartitionSpec("core"),) * n_out,
                  check_rep=False),
        keep_unused=True)

    # replicate a [H, H] array to the concat-of-copies layout [8H, H]
    # on-device: upload 1/8th per core, all_gather over NeuronLink.
    bcast = jax.jit(
        shard_map(lambda ws: jax.lax.all_gather(ws, "core", axis=0,
                                                tiled=True),
                  mesh=mesh, in_specs=PartitionSpec("core"),
                  out_specs=PartitionSpec("core")))

    rt = {
        "jax": jax, "mesh": mesh, "shard": shard,
        "in_names": in_names, "sharded": sharded,
        "bcast": bcast, "pool": ThreadPoolExecutor(2),
    }
    _G["rt"] = rt
    return rt


def _stage_static(rt, expert_w, expert_b):
    """Device-resident f16 weights, cached across calls by content."""
    jax = rt["jax"]
    cached = _G.get("static")
    if cached is not None:
        cw, cb, arrs = cached
        if ((cw is expert_w or np.array_equal(cw, expert_w))
                and (cb is expert_b or np.array_equal(cb, expert_b))):
            return arrs
    w16 = np.ascontiguousarray(expert_w.astype(np.float16))      # [H, H]
    try:
        w_dev = rt["bcast"](jax.device_put(w16, rt["shard"]))
        w_dev.block_until_ready()
    except Exception:
        w_dev = jax.device_put(
            np.ascontiguousarray(np.tile(w16, (NCORES, 1))), rt["shard"])
    bvec16 = np.tile(expert_b.astype(np.float16).reshape(1, H), (NCORES, 1))
    ident = np.tile(np.eye(128, dtype=np.float16), (NCORES, 1))
    arrs = {
        "w": w_dev,
        "bvec": jax.device_put(np.ascontiguousarray(bvec16), rt["shard"]),
        "ident": jax.device_put(np.ascontiguousarray(ident), rt["shard"]),
    }
    _G["static"] = (expert_w, expert_b, arrs)
    return arrs


def kernel(x, router_w, router_b, expert_w, expert_b):
    import time
    last = None
    for attempt in range(3):
        try:
            return _kernel_impl(x, router_w, router_b, expert_w, expert_b)
        except Exception as e:
            last = e
            _reset_jax()
            if attempt == 1:
                # wedged devices have been observed to need a long
                # runtime-side reset window before recovering
                time.sleep(25.0)
    raise last


def _reset_jax():
    """The axon-tunneled devices occasionally come up wedged
    (NRT_EXEC_UNIT_UNRECOVERABLE) on the first touch from a fresh
    process; clearing the PJRT client and rebuilding (patiently)
    recovers."""
    import time

    import jax
    _G.clear()
    time.sleep(5.0)
    try:
        jax.clear_caches()
    except Exception:
        pass
    try:
        jax.extend.backend.clear_backends()
    except Exception:
        try:
            jax.clear_backends()
        except Exception:
            pass


def _kernel_impl(x, router_w, router_b, expert_w, expert_b):
    rt = _runtime()
    jax = rt["jax"]
    x = np.asarray(x, dtype=np.float32)
    xf = np.ascontiguousarray(x.reshape(BS, H))
    xc = xf.reshape(NCORES, NCHUNK, CTPC, H)
    nrow = (CMT * 512 + H - 1) // H

    statics = _stage_static(rt, np.asarray(expert_w, np.float32),
                            np.asarray(expert_b, np.float32))

    # per-token int8 quantization, chunk-streamed so the first H2D starts
    # while the host still quantizes / routes (single CPU, async puts).
    # Conversion buffers are reused across calls to avoid page faults.
    bufs = _G.get("bufs")
    if bufs is None:
        bufs = {
            "v": np.empty((NCORES, CTPC, H), np.float32),
            "x8": [np.empty((NCORES * CTPC, H), np.int8)
                   for _ in range(NCHUNK)],
            "iv": [np.empty((NCORES, CTPC), np.float16)
                   for _ in range(NCHUNK)],
            "s": np.empty((NCORES, NCHUNK, CTPC), np.float32),
        }
        _G["bufs"] = bufs

    def _quant_put(ch, invc):
        # round-half-up int8 quant via offset-binary: |v| <= 127(1+eps)
        # by construction, so v+128.5 in (1.49, 255.51) fits uint8
        v = bufs["v"]
        np.multiply(xc[:, ch], invc[:, ch, :, None], out=v)
        v += 128.5
        x8 = bufs["x8"][ch]
        x8u = x8.view(np.uint8)
        np.copyto(x8u.reshape(NCORES, CTPC, H), v, casting="unsafe")
        x8u ^= 128
        xd = jax.device_put(x8, rt["shard"])
        iv = bufs["iv"][ch]
        np.copyto(iv, invc[:, ch], casting="same_kind")
        return xd, jax.device_put(iv, rt["shard"])

    def _dispatch(ch, csc, x_dev, invs_dev, outs):
        # cs rides the wire BEFORE the next chunk's 8.4 MB x8 (FIFO
        # queue), so exec ch can start the moment its x8 lands and its
        # RPC latency hides under the remaining uploads.  The D2H is
        # requested via copy_to_host_async — a non-polling prefetch
        # that fires when the exec completes; Python-side waits would
        # otherwise throttle the in-flight uploads ~2x.
        cmat = np.ascontiguousarray(
            csc[:, ch].transpose(0, 2, 1).reshape(NCORES * 128, CMT))
        cs_dev = jax.device_put(cmat, rt["shard"])
        args = {"x8": x_dev, "cs": cs_dev, "invs": invs_dev, **statics}
        out = rt["sharded"](*[args[n] for n in rt["in_names"]])
        try:
            out[0].copy_to_host_async()
        except Exception:
            pass
        outs.append(out)

    s = bufs["s"]
    np.maximum(xf.max(axis=1), -xf.min(axis=1), out=s.reshape(BS))
    np.maximum(s, 1e-30, out=s)
    s /= 127.0
    invc = 1.0 / s

    outs = []
    xd0, ivd0 = _quant_put(0, invc)
    coeff = _routing_coeff(xf, np.asarray(router_w, np.float32),
                           np.asarray(router_b, np.float32))
    s = s.reshape(BS)
    csc = (coeff * s).astype(np.float32).reshape(NCORES, NCHUNK, CMT, 128)
    _dispatch(0, csc, xd0, ivd0, outs)
    for ch in range(1, NCHUNK):
        xd, ivd = _quant_put(ch, invc)
        _dispatch(ch, csc, xd, ivd, outs)
    y = np.empty((NCORES, NCHUNK, CTPC, H), np.float32)
    for ch in range(NCHUNK):
        raw = np.asarray(outs[ch][0]).reshape(NCORES, CTPC + nrow, H)
        h = np.ascontiguousarray(raw[:, CTPC:]).view(np.float32) \
              .reshape(NCORES, 128, CMT)
        hm = h.transpose(0, 2, 1).reshape(NCORES, CTPC)   # token order
        np.multiply(raw[:, :CTPC], hm[:, :, None], out=y[:, ch])
    yf = y.reshape(BS, H)

    # spot-check a few tokens against exact host compute (~2 ms): a
    # degraded tunnel has been observed to silently corrupt the
    # device-resident weights; raising here routes into the reset+
    # re-upload retry instead of returning garbage
    idx = np.array([5, 1033, 2901, 4096, 5333, 8001])
    ref = (xf[idx] @ np.asarray(expert_w, np.float32).reshape(H, H)
           + np.asarray(expert_b, np.float32).reshape(H)) * coeff[idx, None]
    err = np.abs(yf[idx] - ref).max() / max(float(np.abs(ref).max()), 1e-6)
    if not np.isfinite(err) or err > 0.08:
        raise RuntimeError(f"device output failed spot-check: rel {err:.3g}")
    return yf.reshape(4, 2048, H)
